# revision 21
# baseline (speedup 1.0000x reference)
"""Trainium2 Bass kernel for the NeuralMemory (scatter_memory) problem.

Math (per batch b, all derived in closed form from the reference):
  keys/vals/q = l2norm_over_T(silu(x @ W.T))          (feature-major)
  a    = W1 @ keys^T ; h = silu(a) ; sp = silu'(a)
  d    = pred - vals = (W2 @ h^T) - vals^T            (raw, no 2/E yet)
  e    = (W2^T d)*sp
  ce   = coeff_eff[t]*e ; cd = coeff_eff[t]*d          (coeff_eff = coeff*2/(E*B))
  W1f^T = decay*W1^T + sum_t keys[t]*ce[t]^T   (T-contraction matmul)
  W2f^T = decay*W2^T + sum_t h[t]*cd[t]^T
  b1f  = sum_t ce[t] ; b2f = sum_t cd[t]
  out  = (W2f @ silu(W1f @ q^T + b1f)) + b2f

Layout: "fm" = feature-major packed (128, 512): partition p = e + 64*j,
column t' with t = t' + 512*j.  T-major chunks obtained by PE transposes of
128-col fm slices against an identity.  One batch per NeuronCore (8 cores).
"""

import os

import numpy as np

import concourse.bacc as bacc
import concourse.mybir as mybir
from concourse.tile import TileContext
from concourse.bass_utils import run_bass_kernel_spmd

ALPHA, ETA, THETA = 0.999, 0.6, 0.05
B, T, E, H = 8, 1024, 64, 64
FP = mybir.dt.float32
AF = mybir.ActivationFunctionType
ALU = mybir.AluOpType

# The HW-native Silu / Derivative_silu activation tables crash the device on
# this runtime (NRT unrecoverable), so default to the explicit sigmoid-based
# path (which CoreSim also supports).  Set KERNEL_NATIVE_SILU=1 to try native.
USE_NATIVE_SILU = bool(os.environ.get("KERNEL_NATIVE_SILU"))

_NC_CACHE = {}


def _emit_dual(nc, psum, lhsT_dup, rhs_fm, start=True, stop=True, skip=False):
    """A-type matmul on packed fm operands: two concurrent 64x64 tiles."""
    nc.tensor.matmul(psum[0:64, :], lhsT_dup[0:64, :], rhs_fm[0:64, :],
                     start=start, stop=stop, skip_group_check=skip)
    nc.tensor.matmul(psum[64:128, :], lhsT_dup[64:128, :], rhs_fm[64:128, :],
                     start=start, stop=stop, skip_group_check=skip)


def build_nc(finalize=True, stage=None, bench_iters=1):
    if stage is None:
        stage = int(os.environ.get("KERNEL_STAGE", "9"))
    nc = bacc.Bacc("TRN2", target_bir_lowering=False, debug=False)

    # ONE input blob per core (single DMA => a single DMA semaphore gates all
    # matmul inputs), one output.
    blob_d = nc.declare_dram_parameter("blob", [128, 1096], FP, isOutput=False)
    out_d = nc.declare_dram_parameter("outp", [128, 512], FP, isOutput=True)

    with TileContext(nc) as tc:
        with (
            tc.tile_pool(name="persist", bufs=1) as pp,
            tc.tile_pool(name="rot", bufs=2) as rot,
            tc.tile_pool(name="small", bufs=1) as sm,
            tc.tile_pool(name="psmm", bufs=3, space="PSUM") as psmm,
            tc.tile_pool(name="pstr", bufs=3, space="PSUM") as pstr,
            tc.tile_pool(name="psacc", bufs=1, space="PSUM") as psacc,
        ):
            # ---- load all inputs with one DMA, slice views out of the blob --
            blob_sb = pp.tile([128, 1096], FP, tag="blob_sb", name="blob_sb")
            nc.sync.dma_start(out=blob_sb[:, :], in_=blob_d[:, :])
            x_sb = blob_sb[:, 0:512]
            wt = {}
            for i, nm in enumerate(["kwT", "vwT", "qwT", "w1T", "w2T", "w2d"]):
                wt[nm] = blob_sb[:, 512 + 64 * i:576 + 64 * i]
            I128 = blob_sb[:, 896:1024]
            dI = blob_sb[0:64, 1024:1088]
            coefc = blob_sb[:, 1088:1096]

            ones_row = pp.tile([1, 512], FP, tag="ones_row", name="ones_row")
            nc.vector.memset(ones_row[:, :], 1.0)
            ones_col = pp.tile([128, 1], FP, tag="ones_col", name="ones_col")
            nc.vector.memset(ones_col[:, :], 1.0)

            out_sb = pp.tile([128, 512], FP, tag="out_sb", name="out_sb")

            import contextlib
            _loop = contextlib.ExitStack()
            if bench_iters > 1:
                _loop.enter_context(tc.For_i(0, bench_iters, 1))

            # ---- phase 1: keys / vals / q with l2norm over T ----
            def norm_path(wname, outname):
                ps = psmm.tile([128, 512], FP, tag="mm", name="mm")
                _emit_dual(nc, ps, wt[wname], x_sb)
                silk = rot.tile([128, 512], FP, tag="silk", name="silk")
                if USE_NATIVE_SILU:
                    nc.scalar.activation(silk[:, :], ps[:, :], AF.Silu)
                else:
                    sig = rot.tile([128, 512], FP, tag="sig", name="sig")
                    nc.scalar.activation(sig[:, :], ps[:, :], AF.Sigmoid)
                    nc.vector.tensor_mul(silk[:, :], ps[:, :], sig[:, :])
                sq = rot.tile([128, 512], FP, tag="sq", name="sq")
                sums = sm.tile([128, 1], FP, tag=f"sums_{outname}",
                               name=f"sums_{outname}")
                nc.scalar.activation(sq[:, :], silk[:, :], AF.Square,
                                     accum_out=sums[:, :])
                sh = sm.tile([64, 1], FP, tag=f"sh_{outname}",
                             name=f"sh_{outname}")
                nc.vector.tensor_copy(sh[:, :], sums[64:128, :])
                s2 = sm.tile([64, 1], FP, tag=f"s2_{outname}",
                             name=f"s2_{outname}")
                nc.vector.tensor_add(s2[:, :], sums[0:64, :], sh[:, :])
                sn = sm.tile([64, 1], FP, tag=f"sn_{outname}",
                             name=f"sn_{outname}")
                nc.scalar.sqrt(sn[:, :], s2[:, :])
                rs = sm.tile([64, 1], FP, tag=f"rs_{outname}",
                             name=f"rs_{outname}")
                nc.vector.reciprocal(rs[:, :], sn[:, :])
                rs128 = sm.tile([128, 1], FP, tag=f"rs128_{outname}",
                                name=f"rs128_{outname}")
                nc.vector.tensor_copy(rs128[0:64, :], rs[:, :])
                nc.vector.tensor_copy(rs128[64:128, :], rs[:, :])
                ofm = pp.tile([128, 512], FP, tag=outname, name=outname)
                nc.vector.tensor_scalar_mul(ofm[:, :], silk[:, :], rs128[:, :])
                return ofm

            keys_fm = norm_path("kwT", "keys_fm")
            vals_fm = norm_path("vwT", "vals_fm")
            q_fm = norm_path("qwT", "q_fm")

            if stage >= 2:
                # ---- phase 2: a, h, sp ----
                psA = psmm.tile([128, 512], FP, tag="mm", name="mm")
                _emit_dual(nc, psA, wt["w1T"], keys_fm)
                h_fm = pp.tile([128, 512], FP, tag="h_fm", name="h_fm")
                sp_fm = pp.tile([128, 512], FP, tag="sp_fm", name="sp_fm")
                if USE_NATIVE_SILU:
                    nc.scalar.activation(h_fm[:, :], psA[:, :], AF.Silu)
                    nc.scalar.activation(sp_fm[:, :], psA[:, :],
                                         AF.Derivative_silu)
                else:
                    sigA = rot.tile([128, 512], FP, tag="sig", name="sig")
                    nc.scalar.activation(sigA[:, :], psA[:, :], AF.Sigmoid)
                    nc.vector.tensor_mul(h_fm[:, :], psA[:, :], sigA[:, :])
                    t1 = rot.tile([128, 512], FP, tag="t1", name="t1")
                    nc.scalar.add(t1[:, :], psA[:, :], 1.0)
                    t2 = rot.tile([128, 512], FP, tag="t2", name="t2")
                    nc.vector.tensor_sub(t2[:, :], t1[:, :], h_fm[:, :])
                    nc.vector.tensor_mul(sp_fm[:, :], sigA[:, :], t2[:, :])

                # ---- phase 3: d_raw, e_raw ----
                psP = psmm.tile([128, 512], FP, tag="mm", name="mm")
                _emit_dual(nc, psP, wt["w2T"], h_fm)
                d_raw = pp.tile([128, 512], FP, tag="d_raw", name="d_raw")
                nc.vector.tensor_sub(d_raw[:, :], psP[:, :], vals_fm[:, :])

                psE = psmm.tile([128, 512], FP, tag="mm", name="mm")
                _emit_dual(nc, psE, wt["w2d"], d_raw)
                e_raw = pp.tile([128, 512], FP, tag="e_raw", name="e_raw")
                nc.vector.tensor_mul(e_raw[:, :], psE[:, :], sp_fm[:, :])

            if stage >= 4:
                # ---- phase 4: transposes to T-major chunks ----
                # kh_tr[cc]: cols 0:64 = (keys chunk cc)^T, 64:128 = (h chunk)^T
                # ed_tr[cc]: cols 0:64 = coeff*(e chunk)^T, 64:128 = coeff*(d)^T
                kh_tr = [pp.tile([128, 128], FP, tag=f"kh_tr{cc}",
                                 name=f"kh_tr{cc}") for cc in range(8)]
                ed_tr = [pp.tile([128, 128], FP, tag=f"ed_tr{cc}",
                                 name=f"ed_tr{cc}") for cc in range(8)]
                for c in range(4):
                    sl = slice(128 * c, 128 * (c + 1))
                    pk = pstr.tile([128, 128], FP, tag="tr", name="tr")
                    nc.tensor.matmul(pk[:, :], keys_fm[:, sl], I128,
                                     start=True, stop=True)
                    nc.scalar.copy(kh_tr[c][:, 0:64], pk[:, 0:64])
                    nc.scalar.copy(kh_tr[c + 4][:, 0:64], pk[:, 64:128])
                    ph = pstr.tile([128, 128], FP, tag="tr", name="tr")
                    nc.tensor.matmul(ph[:, :], h_fm[:, sl], I128,
                                     start=True, stop=True)
                    nc.scalar.copy(kh_tr[c][:, 64:128], ph[:, 0:64])
                    nc.scalar.copy(kh_tr[c + 4][:, 64:128], ph[:, 64:128])
                    pe = pstr.tile([128, 128], FP, tag="tr", name="tr")
                    nc.tensor.matmul(pe[:, :], e_raw[:, sl], I128,
                                     start=True, stop=True)
                    nc.scalar.activation(ed_tr[c][:, 0:64], pe[:, 0:64],
                                         AF.Copy, scale=coefc[:, c:c + 1])
                    nc.scalar.activation(ed_tr[c + 4][:, 0:64], pe[:, 64:128],
                                         AF.Copy, scale=coefc[:, 4 + c:5 + c])
                    pd = pstr.tile([128, 128], FP, tag="tr", name="tr")
                    nc.tensor.matmul(pd[:, :], d_raw[:, sl], I128,
                                     start=True, stop=True)
                    nc.scalar.activation(ed_tr[c][:, 64:128], pd[:, 0:64],
                                         AF.Copy, scale=coefc[:, c:c + 1])
                    nc.scalar.activation(ed_tr[c + 4][:, 64:128],
                                         pd[:, 64:128],
                                         AF.Copy, scale=coefc[:, 4 + c:5 + c])

            if stage >= 5:
                # ---- phase 5: T-contraction (B-type) + bias rows ----
                # psB quadrants: [0:64,0:64]    = sum_t keys[t] ce[t]^T (e,h)
                #                [64:128,64:128]= sum_t h[t] cd[t]^T    (h,e)
                psB = psacc.tile([128, 128], FP, tag="psB", name="psB")
                psb = psacc.tile([1, 128], FP, tag="psb", name="psb")
                for cc in range(8):
                    nc.tensor.matmul(psB[:, :], kh_tr[cc][:, :],
                                     ed_tr[cc][:, :], start=(cc == 0),
                                     stop=False, skip_group_check=True)
                    nc.tensor.matmul(psb[:, :], ones_col[:, :],
                                     ed_tr[cc][:, :], start=(cc == 0),
                                     stop=(cc == 7), skip_group_check=True)
                # inject decay * W1T / W2T into the accumulating quadrants
                nc.tensor.matmul(psB[0:64, 0:64], dI, wt["w1T"][0:64, :],
                                 start=False, stop=False,
                                 skip_group_check=True)
                nc.tensor.matmul(psB[64:128, 64:128], dI, wt["w2T"][0:64, :],
                                 start=False, stop=True, skip_group_check=True)

                # ---- phase 6: final fast weights (duplicated for dual-tile) --
                w1fT = pp.tile([128, 64], FP, tag="w1fT", name="w1fT")
                nc.scalar.copy(w1fT[0:64, :], psB[0:64, 0:64])
                nc.vector.tensor_copy(w1fT[64:128, :], psB[0:64, 0:64])
                w2fT = pp.tile([128, 64], FP, tag="w2fT", name="w2fT")
                nc.vector.tensor_copy(w2fT[0:64, :], psB[64:128, 64:128])
                nc.scalar.copy(w2fT[64:128, :], psB[64:128, 64:128])

                b1row = sm.tile([1, 128], FP, tag="b1row", name="b1row")
                nc.scalar.copy(b1row[:, 0:64], psb[:, 0:64])
                nc.scalar.copy(b1row[:, 64:128], psb[:, 0:64])
                b2row = sm.tile([1, 128], FP, tag="b2row", name="b2row")
                nc.scalar.copy(b2row[:, 0:64], psb[:, 64:128])
                nc.scalar.copy(b2row[:, 64:128], psb[:, 64:128])

            if stage >= 7:
                # ---- phase 7: retrieval ----
                psR1 = psmm.tile([128, 512], FP, tag="mm", name="mm")
                nc.tensor.matmul(psR1[:, :], b1row[:, :], ones_row[:, :],
                                 start=True, stop=False, skip_group_check=True)
                nc.tensor.matmul(psR1[0:64, :], w1fT[0:64, :], q_fm[0:64, :],
                                 start=False, stop=False,
                                 skip_group_check=True)
                nc.tensor.matmul(psR1[64:128, :], w1fT[64:128, :],
                                 q_fm[64:128, :], start=False, stop=True,
                                 skip_group_check=True)
                h2_fm = pp.tile([128, 512], FP, tag="h2_fm", name="h2_fm")
                if USE_NATIVE_SILU:
                    nc.scalar.activation(h2_fm[:, :], psR1[:, :], AF.Silu)
                else:
                    sigR = rot.tile([128, 512], FP, tag="sig", name="sig")
                    nc.scalar.activation(sigR[:, :], psR1[:, :], AF.Sigmoid)
                    nc.vector.tensor_mul(h2_fm[:, :], psR1[:, :], sigR[:, :])

                psR2 = psmm.tile([128, 512], FP, tag="mm", name="mm")
                nc.tensor.matmul(psR2[:, :], b2row[:, :], ones_row[:, :],
                                 start=True, stop=False, skip_group_check=True)
                nc.tensor.matmul(psR2[0:64, :], w2fT[0:64, :], h2_fm[0:64, :],
                                 start=False, stop=False,
                                 skip_group_check=True)
                nc.tensor.matmul(psR2[64:128, :], w2fT[64:128, :],
                                 h2_fm[64:128, :], start=False, stop=True,
                                 skip_group_check=True)
                nc.scalar.copy(out_sb[:, :], psR2[:, :])

            # debug output taps for earlier stages
            if stage == 1:
                nc.scalar.copy(out_sb[:, :], q_fm[:, :])
            elif stage in (2, 3):
                nc.scalar.copy(out_sb[:, :], e_raw[:, :])
            elif stage == 4:
                nc.scalar.copy(out_sb[:, 0:128], kh_tr[0][:, :])
                nc.scalar.copy(out_sb[:, 128:256], kh_tr[5][:, :])
                nc.scalar.copy(out_sb[:, 256:384], ed_tr[0][:, :])
                nc.scalar.copy(out_sb[:, 384:512], ed_tr[5][:, :])
            elif stage in (5, 6):
                nc.vector.memset(out_sb[:, :], 0.0)
                nc.scalar.copy(out_sb[:, 0:64], w1fT[:, :])
                nc.scalar.copy(out_sb[:, 64:128], w2fT[:, :])
                nc.scalar.copy(out_sb[0:1, 128:256], b1row[:, :])
                nc.scalar.copy(out_sb[0:1, 256:384], b2row[:, :])

            _loop.close()
            nc.sync.dma_start(out=out_d[:, :], in_=out_sb[:, :])

    if finalize:
        nc.finalize()
    return nc


def _get_nc():
    if "nc" not in _NC_CACHE:
        _NC_CACHE["nc"] = build_nc()
    return _NC_CACHE["nc"]


def _host_inputs(x, Kw, Qw, Vw, W1, b1, W2, b2):
    x = np.asarray(x, np.float32)
    Kw = np.asarray(Kw, np.float32)
    Qw = np.asarray(Qw, np.float32)
    Vw = np.asarray(Vw, np.float32)
    W1 = np.asarray(W1, np.float32)
    W2 = np.asarray(W2, np.float32)

    def dup(a):  # (64,64) -> (128,64) duplicated over partition halves
        return np.concatenate([a, a], axis=0).astype(np.float32)

    decay = np.float64(ALPHA) ** T
    dI = (decay * np.eye(64)).astype(np.float32)
    I128 = np.eye(128, dtype=np.float32)

    n = np.arange(T - 1, -1, -1, dtype=np.float64)
    coeff = -THETA * (ALPHA ** (n + 1.0) - ETA ** (n + 1.0)) / (ALPHA - ETA)
    coeff_eff = (coeff * (2.0 / E) / B).astype(np.float32)
    coefc = np.zeros((128, 8), np.float32)
    for c in range(4):
        coefc[:, c] = coeff_eff[128 * c:128 * (c + 1)]
        coefc[:, 4 + c] = coeff_eff[512 + 128 * c:512 + 128 * (c + 1)]

    consts = np.zeros((128, 584), np.float32)
    off = 0
    for w in [dup(Kw.T), dup(Vw.T), dup(Qw.T), dup(W1.T), dup(W2.T), dup(W2)]:
        consts[:, off:off + 64] = w
        off += 64
    consts[:, 384:512] = I128
    consts[0:64, 512:576] = dI
    consts[:, 576:584] = coefc

    in_maps = []
    for b in range(B):
        z = np.ascontiguousarray(x[b].T)  # (64, 1024)
        xfm = np.concatenate([z[:, :512], z[:, 512:]], axis=0)  # (128, 512)
        blob = np.concatenate([xfm, consts], axis=1)  # (128, 1096)
        in_maps.append({"blob": np.ascontiguousarray(blob)})
    return in_maps


def _unpack(res_list):
    out = np.empty((B, T, E), np.float32)
    for b in range(B):
        o = res_list[b]["outp"]  # (128, 512)
        out[b] = np.concatenate([o[:64, :], o[64:, :]], axis=1).T
    return out


def run(inputs_dict, trace=False):
    nc = _get_nc()
    in_maps = _host_inputs(**inputs_dict)
    r = run_bass_kernel_spmd(nc, in_maps, list(range(B)), trace=trace)
    return _unpack(r.results), r


def kernel(x, Kw, Qw, Vw, W1, b1, W2, b2):
    out, _ = run(dict(x=x, Kw=Kw, Qw=Qw, Vw=Vw, W1=W1, b1=b1, W2=W2, b2=b2))
    return out


def bench(inputs_dict, n_lo=1000, n_hi=11000, reps=8):
    """Estimate per-body HW time by timing device-looped variants.

    Returns ns per body iteration (includes the Tile loop back-edge,
    ~1-2us, so it is an upper bound on the single-shot body time).
    """
    import time
    in_maps = _host_inputs(**inputs_dict)
    times = {}
    for n in (n_lo, n_hi):
        nc = build_nc(bench_iters=n)
        run_bass_kernel_spmd(nc, in_maps, list(range(B)))  # compile+warm
        best = float("inf")
        for _ in range(reps):
            t0 = time.perf_counter()
            run_bass_kernel_spmd(nc, in_maps, list(range(B)))
            best = min(best, time.perf_counter() - t0)
        times[n] = best
    ns = (times[n_hi] - times[n_lo]) / (n_hi - n_lo) * 1e9
    return ns, times


# revision 23
# speedup vs baseline: 1.1762x; 1.1762x over previous
"""Trainium2 Bass kernel for the NeuralMemory (scatter_memory) problem.

Math (per batch b, derived in closed form from the reference):
  keys/vals/q = l2norm_over_T(silu(x @ W.T))          (feature-major)
  a    = W1 @ keys^T ; h = silu(a) ; sp = silu'(a)
  cd   = coeff_eff[t] * ((W2 @ h^T) - vals^T)          (coeff_eff = coeff*2/(E*B))
  ce   = (W2^T @ cd) * sp
  W1f^T = decay*W1^T + sum_t keys[t] ce[t]^T           (T-contraction matmul)
  W2f^T = decay*W2^T + sum_t h[t] cd[t]^T
  b1f  = sum_t ce[t] ; b2f = sum_t cd[t]
  out  = W2f @ silu(W1f @ q^T + b1f) + b2f

Layout: "fm" = feature-major packed (128, 512): partition p = e + 64*j,
column t' with t = t' + 512*j.  T-major chunks via PE transpose-mode against
an identity.  The l2norm scales for keys and q are folded into downstream
matmul stationaries, so the raw silu outputs feed the transposes directly.
One batch per NeuronCore (8 cores).
"""

import os

import numpy as np

import concourse.bacc as bacc
import concourse.mybir as mybir
from concourse.tile import TileContext
from concourse.bass_utils import run_bass_kernel_spmd

ALPHA, ETA, THETA = 0.999, 0.6, 0.05
B, T, E, H = 8, 1024, 64, 64
FP = mybir.dt.float32
I32 = mybir.dt.int32
AF = mybir.ActivationFunctionType
ALU = mybir.AluOpType
MAGIC = 0x5F3759DF

_NC_CACHE = {}

# blobA columns: x (0:512) | kwT vwT qwT w1T w2T w2d (512:896), all dup'd
# blobB columns: I128 (0:128) | coeff_bc (128:640) | dW1T (640:704, rows 0:64)
#                | dW2T (704:768, rows 0:64)
BLOBA_COLS = 896
BLOBB_COLS = 768


def _emit_dual(nc, psum, lhsT_dup, rhs_fm, start=True, stop=True):
    nc.tensor.matmul(psum[0:64, :], lhsT_dup[0:64, :], rhs_fm[0:64, :],
                     start=start, stop=stop)
    nc.tensor.matmul(psum[64:128, :], lhsT_dup[64:128, :], rhs_fm[64:128, :],
                     start=start, stop=stop)


def build_nc(finalize=True, bench_iters=1):
    nc = bacc.Bacc("TRN2", target_bir_lowering=False, debug=False)

    blobA_d = nc.declare_dram_parameter("blobA", [128, BLOBA_COLS], FP,
                                        isOutput=False)
    blobB_d = nc.declare_dram_parameter("blobB", [128, BLOBB_COLS], FP,
                                        isOutput=False)
    out_d = nc.declare_dram_parameter("outp", [128, 512], FP, isOutput=True)

    with TileContext(nc) as tc:
        with (
            tc.tile_pool(name="persist", bufs=1) as pp,
            tc.tile_pool(name="rot", bufs=2) as rot,
            tc.tile_pool(name="small", bufs=1) as sm,
            tc.tile_pool(name="psmm", bufs=3, space="PSUM") as psmm,
            tc.tile_pool(name="pstr", bufs=3, space="PSUM") as pstr,
            tc.tile_pool(name="psacc", bufs=1, space="PSUM") as psacc,
        ):
            blobA = pp.tile([128, BLOBA_COLS], FP, tag="blobA", name="blobA")
            nc.sync.dma_start(out=blobA[:, :], in_=blobA_d[:, :])
            blobB = pp.tile([128, BLOBB_COLS], FP, tag="blobB", name="blobB")
            nc.sync.dma_start(out=blobB[:, :], in_=blobB_d[:, :])

            x_sb = blobA[:, 0:512]
            wt = {}
            for i, nm in enumerate(["kwT", "vwT", "qwT", "w1T", "w2T", "w2d"]):
                wt[nm] = blobA[:, 512 + 64 * i:576 + 64 * i]
            I128 = blobB[:, 0:128]
            coeff_bc = blobB[:, 128:640]
            dW1T = blobB[0:64, 640:704]
            dW2T = blobB[0:64, 704:768]

            # small constants (no DMA deps)
            magic = sm.tile([64, 1], I32, tag="magic", name="magic")
            nc.vector.memset(magic[:, :], MAGIC)
            wrow = pp.tile([128, 512], FP, tag="wrow", name="wrow")
            nc.gpsimd.memset(wrow[:, :], 0.0)
            warm_lhs = sm.tile([128, 1], FP, tag="warm_lhs", name="warm_lhs")
            nc.vector.memset(warm_lhs[:, :], 0.0)

            out_sb = pp.tile([128, 512], FP, tag="out_sb", name="out_sb")

            import contextlib
            _loop = contextlib.ExitStack()
            if bench_iters > 1:
                _loop.enter_context(tc.For_i(0, bench_iters, 1))

            # ---- PE warm-up during the input DMA (ramps the p-state) ----
            pswarm = psmm.tile([128, 512], FP, tag="mm", name="mm")
            for _ in range(4):
                nc.tensor.matmul(pswarm[0:1, :], warm_lhs[:, 0:1], wrow[:, :],
                                 start=True, stop=True)

            def rsqrt64(s2, nm):
                """1/sqrt(s2) on DVE via fast-inverse-sqrt + 3 Newton steps."""
                s2h = sm.tile([64, 1], FP, tag=f"s2h_{nm}", name=f"s2h_{nm}")
                nc.vector.tensor_scalar_mul(s2h[:, :], s2[:, :], 0.5)
                sh1 = sm.tile([64, 1], I32, tag=f"sh1_{nm}", name=f"sh1_{nm}")
                nc.vector.tensor_scalar(
                    out=sh1[:, :], in0=s2[:, :].bitcast(I32), scalar1=1,
                    scalar2=None, op0=ALU.arith_shift_right)
                y0 = sm.tile([64, 1], I32, tag=f"y0_{nm}", name=f"y0_{nm}")
                nc.vector.tensor_sub(y0[:, :], magic[:, :], sh1[:, :])
                y = y0[:, :].bitcast(FP)
                yn = None
                for it in range(3):
                    yy = sm.tile([64, 1], FP, tag=f"yy{it}_{nm}",
                                 name=f"yy{it}_{nm}")
                    nc.vector.tensor_mul(yy[:, :], y, y)
                    nc.vector.tensor_mul(yy[:, :], yy[:, :], s2h[:, :])
                    nc.vector.tensor_scalar(
                        out=yy[:, :], in0=yy[:, :], scalar1=-1.0, scalar2=1.5,
                        op0=ALU.mult, op1=ALU.add)
                    yn = sm.tile([64, 1], FP, tag=f"yn{it}_{nm}",
                                 name=f"yn{it}_{nm}")
                    nc.vector.tensor_mul(yn[:, :], y, yy[:, :])
                    y = yn[:, :]
                return yn

            # ---- phase 1: silu + (folded) l2norm scales ----
            def stream(wname, nm):
                ps = psmm.tile([128, 512], FP, tag="mm", name="mm")
                _emit_dual(nc, ps, wt[wname], x_sb)
                sig = rot.tile([128, 512], FP, tag="sig", name="sig")
                nc.scalar.activation(sig[:, :], ps[:, :], AF.Sigmoid)
                sil = pp.tile([128, 512], FP, tag=f"sil_{nm}", name=f"sil_{nm}")
                nc.vector.tensor_mul(sil[:, :], ps[:, :], sig[:, :])
                sq = rot.tile([128, 512], FP, tag="sq", name="sq")
                sums = sm.tile([128, 1], FP, tag=f"sums_{nm}",
                               name=f"sums_{nm}")
                nc.scalar.activation(sq[:, :], sil[:, :], AF.Square,
                                     accum_out=sums[:, :])
                shh = sm.tile([64, 1], FP, tag=f"shh_{nm}", name=f"shh_{nm}")
                nc.vector.tensor_copy(shh[:, :], sums[64:128, :])
                s2 = sm.tile([64, 1], FP, tag=f"s2_{nm}", name=f"s2_{nm}")
                nc.vector.tensor_add(s2[:, :], sums[0:64, :], shh[:, :])
                rs = rsqrt64(s2, nm)
                return sil, rs

            silk, rs_k = stream("kwT", "k")
            silv, rs_v = stream("vwT", "v")
            silq, rs_q = stream("qwT", "q")

            # vals needs the materialized normalized values
            rs_v128 = sm.tile([128, 1], FP, tag="rs_v128", name="rs_v128")
            nc.vector.tensor_copy(rs_v128[0:64, :], rs_v[:, :])
            nc.vector.tensor_copy(rs_v128[64:128, :], rs_v[:, :])
            vals_fm = pp.tile([128, 512], FP, tag="vals_fm", name="vals_fm")
            nc.vector.tensor_scalar_mul(vals_fm[:, :], silv[:, :],
                                        rs_v128[:, :])

            # keys scale folds into the W1 stationary
            rs_k128 = sm.tile([128, 1], FP, tag="rs_k128", name="rs_k128")
            nc.vector.tensor_copy(rs_k128[0:64, :], rs_k[:, :])
            nc.vector.tensor_copy(rs_k128[64:128, :], rs_k[:, :])
            w1Ts = sm.tile([128, 64], FP, tag="w1Ts", name="w1Ts")
            nc.vector.tensor_scalar_mul(w1Ts[:, :], wt["w1T"], rs_k128[:, :])

            # ---- phase 2: a, h, sp ----
            psA = psmm.tile([128, 512], FP, tag="mm", name="mm")
            _emit_dual(nc, psA, w1Ts[:, :], silk[:, :])
            sigA = rot.tile([128, 512], FP, tag="sig", name="sig")
            nc.scalar.activation(sigA[:, :], psA[:, :], AF.Sigmoid)
            h_fm = pp.tile([128, 512], FP, tag="h_fm", name="h_fm")
            nc.vector.tensor_mul(h_fm[:, :], psA[:, :], sigA[:, :])
            # sp = sigA * ((a + 1) - h)
            t2 = rot.tile([128, 512], FP, tag="t2", name="t2")
            nc.vector.scalar_tensor_tensor(
                out=t2[:, :], in0=psA[:, :], scalar=1.0, in1=h_fm[:, :],
                op0=ALU.add, op1=ALU.subtract)
            sp_fm = pp.tile([128, 512], FP, tag="sp_fm", name="sp_fm")
            nc.vector.tensor_mul(sp_fm[:, :], sigA[:, :], t2[:, :])

            # ---- phase 3: cd, ce ----
            psP = psmm.tile([128, 512], FP, tag="mm", name="mm")
            _emit_dual(nc, psP, wt["w2T"], h_fm[:, :])
            dr = rot.tile([128, 512], FP, tag="dr", name="dr")
            nc.vector.tensor_sub(dr[:, :], psP[:, :], vals_fm[:, :])
            cd_fm = pp.tile([128, 512], FP, tag="cd_fm", name="cd_fm")
            nc.vector.tensor_mul(cd_fm[:, :], dr[:, :], coeff_bc)

            psE = psmm.tile([128, 512], FP, tag="mm", name="mm")
            _emit_dual(nc, psE, wt["w2d"], cd_fm[:, :])
            ce_fm = pp.tile([128, 512], FP, tag="ce_fm", name="ce_fm")
            nc.vector.tensor_mul(ce_fm[:, :], psE[:, :], sp_fm[:, :])

            # ---- bias cols: b1f = sum_t ce, b2f = sum_t cd (DVE reductions) --
            def bias_col(src_fm, nm):
                sums = sm.tile([128, 1], FP, tag=f"bs_{nm}", name=f"bs_{nm}")
                nc.vector.reduce_sum(sums[:, :], src_fm[:, :],
                                     axis=mybir.AxisListType.X)
                shh = sm.tile([64, 1], FP, tag=f"bsh_{nm}", name=f"bsh_{nm}")
                nc.vector.tensor_copy(shh[:, :], sums[64:128, :])
                col = sm.tile([128, 1], FP, tag=f"bcol_{nm}",
                              name=f"bcol_{nm}")
                nc.vector.tensor_add(col[0:64, :], sums[0:64, :], shh[:, :])
                nc.vector.tensor_copy(col[64:128, :], col[0:64, :])
                return col

            b1c = bias_col(ce_fm, "b1")
            b2c = bias_col(cd_fm, "b2")

            # ---- phase 4: transposes to T-major chunks (PE transpose mode) --
            # X_tr[c] (128,128): cols 0:64 = chunk c, 64:128 = chunk c+4
            trs = {}
            for nm, src, eng in (("k", silk, "act"), ("h", h_fm, "act"),
                                 ("e", ce_fm, "dve"), ("d", cd_fm, "dve")):
                tiles = []
                for c in range(4):
                    ps = pstr.tile([128, 128], FP, tag="tr", name="tr")
                    nc.tensor.transpose(ps[:, :],
                                        src[:, 128 * c:128 * (c + 1)], I128)
                    dst = pp.tile([128, 128], FP, tag=f"{nm}_tr{c}",
                                  name=f"{nm}_tr{c}")
                    if eng == "act":
                        nc.scalar.copy(dst[:, :], ps[:, :])
                    else:
                        nc.vector.tensor_copy(dst[:, :], ps[:, :])
                    tiles.append(dst)
                trs[nm] = tiles

            # ---- phase 5: T-contraction into psB (64,128) ----
            # cols 0:64  = sum_t keys_raw[t] ce[t]^T   (Q11', e x h)
            # cols 64:128= sum_t h[t] cd[t]^T          (Q22 , h x e)
            psB = psacc.tile([64, 128], FP, tag="psB", name="psB")
            for cc in range(8):
                c, base = cc % 4, 64 * (cc // 4)
                nc.tensor.matmul(psB[:, 0:64],
                                 trs["k"][c][:, base:base + 64],
                                 trs["e"][c][:, base:base + 64],
                                 start=(cc == 0), stop=(cc == 7),
                                 skip_group_check=True)
                nc.tensor.matmul(psB[:, 64:128],
                                 trs["h"][c][:, base:base + 64],
                                 trs["d"][c][:, base:base + 64],
                                 start=(cc == 0), stop=(cc == 7),
                                 skip_group_check=True)

            # ---- phase 6: final fast weights ----
            # W1fT = (rs_k*rs_q)[e]*Q11' + rs_q[e]*decay*W1T; W2fT = Q22+decay*W2T
            skq = sm.tile([64, 1], FP, tag="skq", name="skq")
            nc.vector.tensor_mul(skq[:, :], rs_k[:, :], rs_q[:, :])
            dW1q = sm.tile([64, 64], FP, tag="dW1q", name="dW1q")
            nc.vector.tensor_scalar_mul(dW1q[:, :], dW1T, rs_q[:, :])
            w1fT = pp.tile([128, 64], FP, tag="w1fT", name="w1fT")
            nc.vector.scalar_tensor_tensor(
                out=w1fT[0:64, :], in0=psB[:, 0:64], scalar=skq[:, :],
                in1=dW1q[:, :], op0=ALU.mult, op1=ALU.add)
            nc.vector.scalar_tensor_tensor(
                out=w1fT[64:128, :], in0=psB[:, 0:64], scalar=skq[:, :],
                in1=dW1q[:, :], op0=ALU.mult, op1=ALU.add)
            w2fT = pp.tile([128, 64], FP, tag="w2fT", name="w2fT")
            nc.vector.scalar_tensor_tensor(
                out=w2fT[0:64, :], in0=psB[:, 64:128], scalar=1.0,
                in1=dW2T, op0=ALU.mult, op1=ALU.add)
            nc.vector.scalar_tensor_tensor(
                out=w2fT[64:128, :], in0=psB[:, 64:128], scalar=1.0,
                in1=dW2T, op0=ALU.mult, op1=ALU.add)

            # ---- phase 7: retrieval ----
            psR1 = psmm.tile([128, 512], FP, tag="mm", name="mm")
            _emit_dual(nc, psR1, w1fT[:, :], silq[:, :])
            sigR = rot.tile([128, 512], FP, tag="sig", name="sig")
            nc.scalar.activation(sigR[:, :], psR1[:, :], AF.Sigmoid,
                                 bias=b1c[:, :])
            h2_fm = pp.tile([128, 512], FP, tag="h2_fm", name="h2_fm")
            nc.vector.scalar_tensor_tensor(
                out=h2_fm[:, :], in0=psR1[:, :], scalar=b1c[:, :],
                in1=sigR[:, :], op0=ALU.add, op1=ALU.mult)

            psR2 = psmm.tile([128, 512], FP, tag="mm", name="mm")
            _emit_dual(nc, psR2, w2fT[:, :], h2_fm[:, :])
            nc.scalar.activation(out_sb[:, :], psR2[:, :], AF.Identity,
                                 bias=b2c[:, :])

            _loop.close()
            nc.sync.dma_start(out=out_d[:, :], in_=out_sb[:, :])

    if finalize:
        nc.finalize()
    return nc


def _get_nc():
    if "nc" not in _NC_CACHE:
        _NC_CACHE["nc"] = build_nc()
    return _NC_CACHE["nc"]


def _host_inputs(x, Kw, Qw, Vw, W1, b1, W2, b2):
    x = np.asarray(x, np.float32)
    Kw = np.asarray(Kw, np.float32)
    Qw = np.asarray(Qw, np.float32)
    Vw = np.asarray(Vw, np.float32)
    W1 = np.asarray(W1, np.float32)
    W2 = np.asarray(W2, np.float32)

    def dup(a):
        return np.concatenate([a, a], axis=0).astype(np.float32)

    decay = np.float64(ALPHA) ** T
    n = np.arange(T - 1, -1, -1, dtype=np.float64)
    coeff = -THETA * (ALPHA ** (n + 1.0) - ETA ** (n + 1.0)) / (ALPHA - ETA)
    coeff_eff = (coeff * (2.0 / E) / B).astype(np.float32)
    # coeff_bc fm-packed: [p=e+64j, t'] = coeff_eff[t' + 512j]
    cb = np.zeros((128, 512), np.float32)
    cb[0:64, :] = coeff_eff[0:512][None, :]
    cb[64:128, :] = coeff_eff[512:1024][None, :]

    constsA = np.zeros((128, 384), np.float32)
    off = 0
    for w in [dup(Kw.T), dup(Vw.T), dup(Qw.T), dup(W1.T), dup(W2.T), dup(W2)]:
        constsA[:, off:off + 64] = w
        off += 64

    blobB = np.zeros((128, BLOBB_COLS), np.float32)
    blobB[:, 0:128] = np.eye(128, dtype=np.float32)
    blobB[:, 128:640] = cb
    blobB[0:64, 640:704] = (decay * W1.T).astype(np.float32)
    blobB[0:64, 704:768] = (decay * W2.T).astype(np.float32)

    in_maps = []
    for b in range(B):
        z = np.ascontiguousarray(x[b].T)  # (64, 1024)
        xfm = np.concatenate([z[:, :512], z[:, 512:]], axis=0)  # (128, 512)
        blobA = np.concatenate([xfm, constsA], axis=1)
        in_maps.append({"blobA": np.ascontiguousarray(blobA), "blobB": blobB})
    return in_maps


def _unpack(res_list):
    out = np.empty((B, T, E), np.float32)
    for b in range(B):
        o = res_list[b]["outp"]  # (128, 512)
        out[b] = np.concatenate([o[:64, :], o[64:, :]], axis=1).T
    return out


def run(inputs_dict, trace=False):
    nc = _get_nc()
    in_maps = _host_inputs(**inputs_dict)
    r = run_bass_kernel_spmd(nc, in_maps, list(range(B)), trace=trace)
    return _unpack(r.results), r


def kernel(x, Kw, Qw, Vw, W1, b1, W2, b2):
    out, _ = run(dict(x=x, Kw=Kw, Qw=Qw, Vw=Vw, W1=W1, b1=b1, W2=W2, b2=b2))
    return out


def bench(inputs_dict, n_lo=1000, n_hi=11000, reps=8):
    """Estimate per-body HW time via device-looped variants (includes the
    ~1-2us Tile loop back-edge, so an upper bound on single-shot time)."""
    import time
    in_maps = _host_inputs(**inputs_dict)
    times = {}
    for n in (n_lo, n_hi):
        nc = build_nc(bench_iters=n)
        run_bass_kernel_spmd(nc, in_maps, list(range(B)))  # compile+warm
        best = float("inf")
        for _ in range(reps):
            t0 = time.perf_counter()
            run_bass_kernel_spmd(nc, in_maps, list(range(B)))
            best = min(best, time.perf_counter() - t0)
        times[n] = best
    ns = (times[n_hi] - times[n_lo]) / (n_hi - n_lo) * 1e9
    return ns, times
